# revision 11
# baseline (speedup 1.0000x reference)
"""Diagonally-masked self-attention on 8 trn2 NeuronCores.

Problem: x[4,2048,512], per-head attention (H=8, D=64) with the DIAGONAL
masked out of the softmax, then output projection.

Sharding (per sharding_hint): data-parallel over batch x tensor-parallel
over heads.  Core c handles batch b=c//2 and head group g=c%2 (4 heads:
global heads 4g..4g+3, i.e. rows g*256:(g+1)*256 of wq/wk/wv and cols of
wo).  Each core holds the full sequence, so the diagonal mask needs no
communication.  Each core produces a partial output [2048,512] (its 4
heads' contribution through wo); the host unshards by summing the two
partials per batch (wo in-dim is split by head => gather is a sum).

Kernel layout choices:
 - scores are computed TRANSPOSED: ST[s,i] = k^T q (contraction over d=64
   on partitions) so that after exp, the PV matmul consumes exp(ST) tiles
   directly (contraction over s on partitions) with no PE transposes.
 - softmax denominator: a row of ones appended to V (lhsT = [v | 1],
   M=65) so PSUM row 64 accumulates colsum(exp(ST)) for free.
 - diagonal mask: for s-tile st, only the i-block [st*128,(st+1)*128)
   intersects the diagonal; that exp block is multiplied by a constant
   (1 - I) "hole" mask.
 - no max-subtraction: scores ~ N(0,1) (scale 1/8 folded into wq on the
   host), exp never overflows, exp(-1e4)=0 matches the reference mask.
 - matmuls run as float32r (full-speed fp32 mode on the PE).
"""

import numpy as np

B, L, DIM, H, D = 4, 2048, 512, 8, 64
HPC = 4  # heads per core
N_CORES = 8
SCALE = D ** -0.5

DT_NAME = "f32r"  # "f32r" | "bf16" | "f32"
_CACHE = {}


def _build_nc(dt_name=DT_NAME):
    import concourse.bass as bass
    import concourse.mybir as mybir
    from concourse import bacc
    from concourse.tile import TileContext

    f32 = mybir.dt.float32
    f32r = mybir.dt.float32r
    CDT = {"f32r": mybir.dt.float32r, "bf16": mybir.dt.bfloat16,
           "f32": mybir.dt.float32}[dt_name]
    EXP = mybir.ActivationFunctionType.Exp

    KT = DIM // 128     # 4 contraction tiles over DIM
    ST = L // 128       # 16 s-tiles
    IH = 2              # i halves (PSUM budget)
    IHW = L // IH       # 1024
    NC2 = IHW // 512    # 512-wide chunks per i-half

    nc = bacc.Bacc("TRN2", target_bir_lowering=False, debug=False,
                   num_devices=N_CORES)

    def _msf(ap):
        # memset has no float32r ISA encoding; write through a f32 view
        return ap.bitcast(f32) if CDT == f32r else ap
    xT_d = nc.declare_dram_parameter("xT", [DIM, L], CDT, isOutput=False)
    wqT_d = nc.declare_dram_parameter("wqT", [DIM, HPC * D], CDT, isOutput=False)
    wkT_d = nc.declare_dram_parameter("wkT", [DIM, HPC * D], CDT, isOutput=False)
    wvT_d = nc.declare_dram_parameter("wvT", [DIM, HPC * D], CDT, isOutput=False)
    woT_d = nc.declare_dram_parameter("woT", [HPC * D, DIM], CDT, isOutput=False)
    hole_d = nc.declare_dram_parameter("hole", [128, 128], CDT, isOutput=False)
    part_d = nc.declare_dram_parameter("part", [L, DIM], f32, isOutput=True)

    with TileContext(nc) as tc, \
         nc.allow_low_precision(reason="attention weights/operands rounded to "
                                "bf16/f32r by design; accumulation stays f32"):
        with tc.tile_pool(name="const", bufs=1) as cp:
            # ---- load inputs ----
            xT = []
            for k in range(KT):
                t = cp.tile([128, L], CDT, name=f"xT{k}")
                nc.sync.dma_start(out=t[:], in_=xT_d[k * 128:(k + 1) * 128, :])
                xT.append(t)
            wT = {}
            for nm, d in (("q", wqT_d), ("k", wkT_d), ("v", wvT_d)):
                wT[nm] = []
                for k in range(KT):
                    t = cp.tile([128, HPC * D], CDT, name=f"w{nm}T{k}")
                    nc.sync.dma_start(out=t[:], in_=d[k * 128:(k + 1) * 128, :])
                    wT[nm].append(t)
            woT = []
            for h in range(HPC):
                t = cp.tile([64, DIM], CDT, name=f"woT{h}")
                nc.sync.dma_start(out=t[:], in_=woT_d[h * 64:(h + 1) * 64, :])
                woT.append(t)
            hole = cp.tile([128, 128], CDT, name="hole")
            nc.sync.dma_start(out=hole[:], in_=hole_d[:, :])
            ones1 = cp.tile([1, 64], CDT, name="ones1")
            nc.gpsimd.memset(_msf(ones1[:]), 1.0)

            # ---- persistent intermediates ----
            # qT/kT: [256,2048] as 2 tiles of [128(=2 heads),2048]
            qT = [cp.tile([128, L], CDT, name=f"qT{i}") for i in range(2)]
            kT = [cp.tile([128, L], CDT, name=f"kT{i}") for i in range(2)]
            # v_aug: per s-tile [128, 4*65]; head h at cols h*65..h*65+64,
            # col h*65+64 = 1.0 (colsum row)
            vaug = [cp.tile([128, HPC * 65], CDT, name=f"vaug{s}") for s in range(ST)]
            # normalized per-head attention output, transposed: [64, 2048]
            yT = [cp.tile([64, L], CDT, name=f"yT{h}") for h in range(HPC)]

            # ---- projections ----
            with tc.tile_pool(name="pj", bufs=4, space="PSUM") as pjp:
                for nm, dst in (("q", qT), ("k", kT)):
                    for mt in range(2):  # head pair
                        for ncc in range(L // 512):
                            ps = pjp.tile([128, 512], f32, tag="pj")
                            for k in range(KT):
                                nc.tensor.matmul(
                                    ps[:],
                                    lhsT=wT[nm][k][:, mt * 128:(mt + 1) * 128],
                                    rhs=xT[k][:, ncc * 512:(ncc + 1) * 512],
                                    start=(k == 0), stop=(k == KT - 1),
                                )
                            nc.vector.tensor_copy(
                                dst[mt][:, ncc * 512:(ncc + 1) * 512], ps[:])
                for st in range(ST):
                    nc.gpsimd.memset(_msf(vaug[st][:]), 1.0)
                    ps = pjp.tile([128, HPC * D], f32, tag="pj")
                    for k in range(KT):
                        nc.tensor.matmul(
                            ps[:],
                            lhsT=xT[k][:, st * 128:(st + 1) * 128],
                            rhs=wT["v"][k][:],
                            start=(k == 0), stop=(k == KT - 1),
                        )
                    # strided copy: psum [128,(h d)] -> vaug cols h*65..h*65+63
                    nc.vector.tensor_copy(
                        vaug[st].rearrange("p (h e) -> p h e", e=65)[:, :, 0:64],
                        ps.rearrange("p (h e) -> p h e", e=64),
                    )

            # ---- attention ----
            with tc.tile_pool(name="ss", bufs=2, space="PSUM") as ssp, \
                 tc.tile_pool(name="pv", bufs=2, space="PSUM") as pvp, \
                 tc.tile_pool(name="ex", bufs=3) as ep, \
                 tc.tile_pool(name="sm", bufs=2) as smp:
                for h in range(HPC):
                    tq = h // 2          # qT/kT tile index
                    pb = (h % 2) * 64    # base partition within tile
                    for ih in range(IH):
                        pv = pvp.tile([65, IHW], f32, tag="pv")
                        for st in range(ST):
                            ss = ssp.tile([128, IHW], f32, tag="ss")
                            for c in range(NC2):
                                io = ih * IHW + c * 512
                                nc.tensor.matmul(
                                    ss[:, c * 512:(c + 1) * 512],
                                    lhsT=kT[tq][pb:pb + 64, st * 128:(st + 1) * 128],
                                    rhs=qT[tq][pb:pb + 64, io:io + 512],
                                    start=True, stop=True,
                                )
                            ex = ep.tile([128, IHW], CDT, tag="ex")
                            nc.scalar.activation(ex[:], ss[:], EXP)
                            if st // 8 == ih:  # diagonal block in this i-half
                                off = st * 128 - ih * IHW
                                nc.vector.tensor_mul(
                                    ex[:, off:off + 128], ex[:, off:off + 128], hole[:])
                            for c in range(NC2):
                                nc.tensor.matmul(
                                    pv[:, c * 512:(c + 1) * 512],
                                    lhsT=vaug[st][:, h * 65:h * 65 + 65],
                                    rhs=ex[:, c * 512:(c + 1) * 512],
                                    start=(st == 0), stop=(st == ST - 1),
                                )
                        # epilogue: yT[h][:, ih] = pv[0:64] / colsum.
                        # colsum reciprocal is broadcast across partitions
                        # via a K=1 PE outer product (ones[64] x rec).
                        rec = smp.tile([1, IHW], CDT, tag="rec")
                        nc.vector.reciprocal(rec[:], pv[64:65, :])
                        pr = ssp.tile([64, IHW], f32, tag="ss")
                        for c in range(NC2):
                            nc.tensor.matmul(
                                pr[:, c * 512:(c + 1) * 512],
                                lhsT=ones1[0:1, :],
                                rhs=rec[0:1, c * 512:(c + 1) * 512],
                                start=True, stop=True,
                            )
                        recB = smp.tile([64, IHW], f32, tag="recB")
                        nc.vector.tensor_copy(recB[:], pr[:])
                        nc.vector.tensor_mul(
                            yT[h][:, ih * IHW:(ih + 1) * IHW], pv[0:64, :], recB[:])

            # ---- output projection (partial: this core's 4 heads) ----
            with tc.tile_pool(name="op", bufs=2, space="PSUM") as opp, \
                 tc.tile_pool(name="ob", bufs=2) as obp:
                for it in range(L // 128):
                    ps = opp.tile([128, DIM], f32, tag="op")
                    for h in range(HPC):
                        nc.tensor.matmul(
                            ps[:],
                            lhsT=yT[h][:, it * 128:(it + 1) * 128],
                            rhs=woT[h][:],
                            start=(h == 0), stop=(h == HPC - 1),
                        )
                    ob = obp.tile([128, DIM], f32, tag="ob")
                    nc.vector.tensor_copy(ob[:], ps[:])
                    nc.sync.dma_start(
                        out=part_d[it * 128:(it + 1) * 128, :], in_=ob[:])
    nc.compile()
    return nc


def _np_cdt():
    if DT_NAME == "bf16":
        import ml_dtypes
        return ml_dtypes.bfloat16
    return np.float32


def _get_nc():
    if "nc" not in _CACHE:
        _CACHE["nc"] = _build_nc()
    return _CACHE["nc"]


def _make_in_maps(x, wq, wk, wv, wo):
    x = np.asarray(x, np.float32)
    wq = np.asarray(wq, np.float32)
    wk = np.asarray(wk, np.float32)
    wv = np.asarray(wv, np.float32)
    wo = np.asarray(wo, np.float32)
    hole = (1.0 - np.eye(128)).astype(np.float32)
    in_maps = []
    for c in range(N_CORES):
        b, g = c // 2, c % 2
        hs = slice(g * HPC * D, (g + 1) * HPC * D)
        cdt = _np_cdt()
        in_maps.append({
            "xT": np.ascontiguousarray(x[b].T).astype(cdt),
            "wqT": np.ascontiguousarray((wq[hs] * SCALE).T).astype(cdt),
            "wkT": np.ascontiguousarray(wk[hs].T).astype(cdt),
            "wvT": np.ascontiguousarray(wv[hs].T).astype(cdt),
            "woT": np.ascontiguousarray(wo[:, hs].T).astype(cdt),
            "hole": hole.astype(cdt),
        })
    return in_maps


def _unshard(results):
    out = np.empty((B, L, DIM), np.float32)
    for b in range(B):
        out[b] = results[2 * b]["part"] + results[2 * b + 1]["part"]
    return out


def kernel(x, wq, wk, wv, wo):
    from concourse.bass_utils import run_bass_kernel_spmd
    nc = _get_nc()
    in_maps = _make_in_maps(x, wq, wk, wv, wo)
    res = run_bass_kernel_spmd(nc, in_maps, list(range(N_CORES)))
    return _unshard(res.results)


# revision 15
# speedup vs baseline: 1.2237x; 1.2237x over previous
"""Diagonally-masked self-attention on 8 trn2 NeuronCores.

Problem: x[4,2048,512], per-head attention (H=8, D=64) with the DIAGONAL
masked out of the softmax, then output projection.

Sharding (per sharding_hint): data-parallel over batch x tensor-parallel
over heads.  Core c handles batch b=c//2 and head group g=c%2 (4 heads:
global heads 4g..4g+3, i.e. rows g*256:(g+1)*256 of wq/wk/wv and cols of
wo).  Each core holds the full sequence, so the diagonal mask needs no
communication.  Each core produces a partial output [2048,512] (its 4
heads' contribution through wo); the host unshards by summing the two
partials per batch (wo in-dim is split by head => gather is a sum).

Kernel layout choices:
 - scores are computed TRANSPOSED: ST[s,i] = k^T q (contraction over d=64
   on partitions) so that after exp, the PV matmul consumes exp(ST) tiles
   directly (contraction over s on partitions) with no PE transposes.
 - softmax denominator: a row of ones appended to V (lhsT = [v | 1],
   M=65) so PSUM row 64 accumulates colsum(exp(ST)) for free.
 - diagonal mask: for s-tile st, only the i-block [st*128,(st+1)*128)
   intersects the diagonal; that exp block is multiplied by a constant
   (1 - I) "hole" mask.
 - no max-subtraction: scores ~ N(0,1) (scale 1/8 folded into wq on the
   host), exp never overflows, exp(-1e4)=0 matches the reference mask.
 - matmuls run as float32r (full-speed fp32 mode on the PE).
"""

import numpy as np

B, L, DIM, H, D = 4, 2048, 512, 8, 64
HPC = 4  # heads per core
N_CORES = 8
SCALE = D ** -0.5

DT_NAME = "bf16"  # "f32r" | "bf16" | "f32"
_CACHE = {}


def _build_nc(dt_name=DT_NAME):
    import concourse.bass as bass
    import concourse.mybir as mybir
    from concourse import bacc
    from concourse.tile import TileContext

    f32 = mybir.dt.float32
    f32r = mybir.dt.float32r
    CDT = {"f32r": mybir.dt.float32r, "bf16": mybir.dt.bfloat16,
           "f32": mybir.dt.float32}[dt_name]
    EXP = mybir.ActivationFunctionType.Exp

    KT = DIM // 128     # 4 contraction tiles over DIM
    ST = L // 128       # 16 s-tiles
    IH = 2              # i halves (PSUM budget)
    IHW = L // IH       # 1024
    NC2 = IHW // 512    # 512-wide chunks per i-half

    nc = bacc.Bacc("TRN2", target_bir_lowering=False, debug=False,
                   num_devices=N_CORES)

    def _msf(ap):
        # memset has no float32r ISA encoding; write through a f32 view
        return ap.bitcast(f32) if CDT == f32r else ap
    xT_d = nc.declare_dram_parameter("xT", [DIM, L], CDT, isOutput=False)
    wqT_d = nc.declare_dram_parameter("wqT", [DIM, HPC * D], CDT, isOutput=False)
    wkT_d = nc.declare_dram_parameter("wkT", [DIM, HPC * D], CDT, isOutput=False)
    wvT_d = nc.declare_dram_parameter("wvT", [DIM, HPC * D], CDT, isOutput=False)
    woT_d = nc.declare_dram_parameter("woT", [HPC * D, DIM], CDT, isOutput=False)
    hole_d = nc.declare_dram_parameter("hole", [128, 128], CDT, isOutput=False)
    part_d = nc.declare_dram_parameter("part", [L, DIM], f32, isOutput=True)

    with TileContext(nc) as tc, \
         nc.allow_low_precision(reason="attention weights/operands rounded to "
                                "bf16/f32r by design; accumulation stays f32"):
        with tc.tile_pool(name="const", bufs=1) as cp:
            # ---- load inputs ----
            xT = []
            for k in range(KT):
                t = cp.tile([128, L], CDT, name=f"xT{k}")
                nc.sync.dma_start(out=t[:], in_=xT_d[k * 128:(k + 1) * 128, :])
                xT.append(t)
            wT = {}
            for nm, d in (("q", wqT_d), ("k", wkT_d), ("v", wvT_d)):
                wT[nm] = []
                for k in range(KT):
                    t = cp.tile([128, HPC * D], CDT, name=f"w{nm}T{k}")
                    nc.sync.dma_start(out=t[:], in_=d[k * 128:(k + 1) * 128, :])
                    wT[nm].append(t)
            woT = []
            for h in range(HPC):
                t = cp.tile([64, DIM], CDT, name=f"woT{h}")
                nc.sync.dma_start(out=t[:], in_=woT_d[h * 64:(h + 1) * 64, :])
                woT.append(t)
            hole = cp.tile([128, 128], CDT, name="hole")
            nc.sync.dma_start(out=hole[:], in_=hole_d[:, :])
            ones1 = cp.tile([1, 64], CDT, name="ones1")
            nc.gpsimd.memset(_msf(ones1[:]), 1.0)

            # ---- persistent intermediates ----
            # qT/kT: [256,2048] as 2 tiles of [128(=2 heads),2048]
            qT = [cp.tile([128, L], CDT, name=f"qT{i}") for i in range(2)]
            kT = [cp.tile([128, L], CDT, name=f"kT{i}") for i in range(2)]
            # v_aug: per s-tile [128, 4*65]; head h at cols h*65..h*65+64,
            # col h*65+64 = 1.0 (colsum row)
            vaug = [cp.tile([128, HPC * 65], CDT, name=f"vaug{s}") for s in range(ST)]
            # normalized per-head attention output, transposed: [64, 2048]
            yT = [cp.tile([64, L], CDT, name=f"yT{h}") for h in range(HPC)]

            # ---- projections ----
            with tc.tile_pool(name="pj", bufs=4, space="PSUM") as pjp:
                for nm, dst in (("q", qT), ("k", kT)):
                    for mt in range(2):  # head pair
                        for ncc in range(L // 512):
                            ps = pjp.tile([128, 512], f32, tag="pj")
                            for k in range(KT):
                                nc.tensor.matmul(
                                    ps[:],
                                    lhsT=wT[nm][k][:, mt * 128:(mt + 1) * 128],
                                    rhs=xT[k][:, ncc * 512:(ncc + 1) * 512],
                                    start=(k == 0), stop=(k == KT - 1),
                                )
                            nc.vector.tensor_copy(
                                dst[mt][:, ncc * 512:(ncc + 1) * 512], ps[:])
                for st in range(ST):
                    nc.gpsimd.memset(_msf(vaug[st][:]), 1.0)
                    ps = pjp.tile([128, HPC * D], f32, tag="pj")
                    for k in range(KT):
                        nc.tensor.matmul(
                            ps[:],
                            lhsT=xT[k][:, st * 128:(st + 1) * 128],
                            rhs=wT["v"][k][:],
                            start=(k == 0), stop=(k == KT - 1),
                        )
                    # strided copy: psum [128,(h d)] -> vaug cols h*65..h*65+63
                    nc.vector.tensor_copy(
                        vaug[st].rearrange("p (h e) -> p h e", e=65)[:, :, 0:64],
                        ps.rearrange("p (h e) -> p h e", e=64),
                    )

            # ---- attention ----
            with tc.tile_pool(name="ss", bufs=2, space="PSUM") as ssp, \
                 tc.tile_pool(name="pv", bufs=2, space="PSUM") as pvp, \
                 tc.tile_pool(name="ex", bufs=3) as ep, \
                 tc.tile_pool(name="sm", bufs=2) as smp:
                for h in range(HPC):
                    tq = h // 2          # qT/kT tile index
                    pb = (h % 2) * 64    # base partition within tile
                    # two interleaved streams (i-halves) deepen the PE
                    # pipeline: while one stream waits on exp, the other
                    # stream's matmuls keep the PE busy
                    pv2 = [pvp.tile([65, IHW], f32, tag="pv", name=f"pv{h}_{i}")
                           for i in range(IH)]
                    for st in range(ST):
                        for ih in range(IH):
                            pv = pv2[ih]
                            ss = ssp.tile([128, IHW], f32, tag="ss")
                            for c in range(NC2):
                                io = ih * IHW + c * 512
                                nc.tensor.matmul(
                                    ss[:, c * 512:(c + 1) * 512],
                                    lhsT=kT[tq][pb:pb + 64, st * 128:(st + 1) * 128],
                                    rhs=qT[tq][pb:pb + 64, io:io + 512],
                                    start=True, stop=True,
                                )
                            ex = ep.tile([128, IHW], CDT, tag="ex")
                            nc.scalar.activation(ex[:], ss[:], EXP)
                            if st // 8 == ih:  # diagonal block in this i-half
                                off = st * 128 - ih * IHW
                                nc.vector.tensor_mul(
                                    ex[:, off:off + 128], ex[:, off:off + 128], hole[:])
                            for c in range(NC2):
                                nc.tensor.matmul(
                                    pv[:, c * 512:(c + 1) * 512],
                                    lhsT=vaug[st][:, h * 65:h * 65 + 65],
                                    rhs=ex[:, c * 512:(c + 1) * 512],
                                    start=(st == 0), stop=(st == ST - 1),
                                )
                    for ih in range(IH):
                        # epilogue: yT[h][:, ih] = pv[0:64] / colsum.
                        # reciprocal of a [1,1024] single-partition row costs
                        # ~6.5us on DVE; bounce it through a [128,8] layout
                        # via DMA so all partitions work (~50ns), then
                        # broadcast across partitions with a K=1 PE outer
                        # product (ones[64] x rec).
                        pv = pv2[ih]
                        csum = smp.tile([1, IHW], f32, tag="csum")
                        nc.vector.tensor_copy(csum[:], pv[64:65, :])
                        c128 = smp.tile([128, IHW // 128], f32, tag="c128")
                        nc.sync.dma_start(out=c128[:], in_=csum[:])
                        r128 = smp.tile([128, IHW // 128], CDT, tag="r128")
                        nc.vector.reciprocal(r128[:], c128[:])
                        rec = smp.tile([1, IHW], CDT, tag="rec")
                        nc.sync.dma_start(out=rec[:], in_=r128[:])
                        pr = ssp.tile([64, IHW], f32, tag="ss")
                        for c in range(NC2):
                            nc.tensor.matmul(
                                pr[:, c * 512:(c + 1) * 512],
                                lhsT=ones1[0:1, :],
                                rhs=rec[0:1, c * 512:(c + 1) * 512],
                                start=True, stop=True,
                            )
                        recB = smp.tile([64, IHW], f32, tag="recB")
                        nc.vector.tensor_copy(recB[:], pr[:])
                        nc.vector.tensor_mul(
                            yT[h][:, ih * IHW:(ih + 1) * IHW], pv[0:64, :], recB[:])

            # ---- output projection (partial: this core's 4 heads) ----
            with tc.tile_pool(name="op", bufs=2, space="PSUM") as opp, \
                 tc.tile_pool(name="ob", bufs=2) as obp:
                for it in range(L // 128):
                    ps = opp.tile([128, DIM], f32, tag="op")
                    for h in range(HPC):
                        nc.tensor.matmul(
                            ps[:],
                            lhsT=yT[h][:, it * 128:(it + 1) * 128],
                            rhs=woT[h][:],
                            start=(h == 0), stop=(h == HPC - 1),
                        )
                    ob = obp.tile([128, DIM], f32, tag="ob")
                    nc.vector.tensor_copy(ob[:], ps[:])
                    nc.sync.dma_start(
                        out=part_d[it * 128:(it + 1) * 128, :], in_=ob[:])
    nc.compile()
    return nc


def _np_cdt():
    if DT_NAME == "bf16":
        import ml_dtypes
        return ml_dtypes.bfloat16
    return np.float32


def _get_nc():
    if "nc" not in _CACHE:
        _CACHE["nc"] = _build_nc()
    return _CACHE["nc"]


def _make_in_maps(x, wq, wk, wv, wo):
    x = np.asarray(x, np.float32)
    wq = np.asarray(wq, np.float32)
    wk = np.asarray(wk, np.float32)
    wv = np.asarray(wv, np.float32)
    wo = np.asarray(wo, np.float32)
    hole = (1.0 - np.eye(128)).astype(np.float32)
    in_maps = []
    for c in range(N_CORES):
        b, g = c // 2, c % 2
        hs = slice(g * HPC * D, (g + 1) * HPC * D)
        cdt = _np_cdt()
        in_maps.append({
            "xT": np.ascontiguousarray(x[b].T).astype(cdt),
            "wqT": np.ascontiguousarray((wq[hs] * SCALE).T).astype(cdt),
            "wkT": np.ascontiguousarray(wk[hs].T).astype(cdt),
            "wvT": np.ascontiguousarray(wv[hs].T).astype(cdt),
            "woT": np.ascontiguousarray(wo[:, hs].T).astype(cdt),
            "hole": hole.astype(cdt),
        })
    return in_maps


def _unshard(results):
    out = np.empty((B, L, DIM), np.float32)
    for b in range(B):
        out[b] = results[2 * b]["part"] + results[2 * b + 1]["part"]
    return out


def kernel(x, wq, wk, wv, wo):
    from concourse.bass_utils import run_bass_kernel_spmd
    nc = _get_nc()
    in_maps = _make_in_maps(x, wq, wk, wv, wo)
    res = run_bass_kernel_spmd(nc, in_maps, list(range(N_CORES)))
    return _unshard(res.results)
